# revision 3
# baseline (speedup 1.0000x reference)
"""BitLinear v5: fp8(e3m4) prescaled weights, transpose-free, dequant-free.

Host ships signsT pre-transposed and prescaled by scale*64, quantized to
fp8 e3m4 (4 mantissa bits -> ~1.5e-2 rel err, under the 2e-2 gate); the
1/64 is folded into x, which ships pre-transposed in bf16. Device does
per-block accumulating matmuls (lhsT = fp8 sign tile [128k, 128o],
rhs = bf16 xT [128k, 32b]) into psum [128, 32b]:
  yT[o,b] = sum_g (s*scale*64)[o,g].T @ (x/64)T[g,b]
Mixed fp8xbf16 matmul is legal on trn2 (both upcast to fp22; only fp32
must match). ~5.9MB HBM traffic per core.

v5 vs v4: o-shard padded 1376->1408 (11 full blocks, FWL everywhere),
graded DMA chunks (small first so PE starts early), descriptor generation
alternated across the sync/scalar HWDGE rings, y output split in pieces
that overlap tail compute.
"""

import numpy as np

BATCH = 32
IN_F = 4096
OUT_F = 11008
GROUP = 128
N_GROUPS = IN_F // GROUP  # 32
N_CORES = 8
O_SHARD = OUT_F // N_CORES  # 1376
O_PAD = 1408  # 11 full blocks of 128
N_BLOCKS = O_PAD // 128  # 11
CHUNK_BLOCKS = [1, 1, 2, 2, 2, 3]  # graded: first data lands fast
IMG_F = N_GROUPS * O_PAD  # 45056 free bytes per partition (fp8)
W_RESCALE = 64.0  # lift scales into e3m4 normal range [0.25, 15.5)
Y_PIECES = [(0, 4), (4, 8), (8, 11)]  # y output DMA split (block ranges)

_nc_cache = []


def build_nc():
    import concourse.bacc as bacc
    import concourse.mybir as mybir
    import concourse.tile as tile

    f32 = mybir.dt.float32
    bf16 = mybir.dt.bfloat16
    f8 = mybir.dt.float8e3

    nc = bacc.Bacc(None, target_bir_lowering=False)
    xT_d = nc.dram_tensor("xT", [128, N_GROUPS * BATCH], bf16, kind="ExternalInput")
    sT_d = nc.dram_tensor("signsT", [128, IMG_F], f8, kind="ExternalInput")
    y_d = nc.dram_tensor("y", [128, N_BLOCKS * BATCH], f32, kind="ExternalOutput")

    with tile.TileContext(nc) as tc:
        with tc.tile_pool(name="const", bufs=1) as const, tc.tile_pool(
            name="psum", bufs=1, space="PSUM"
        ) as psum:
            xT = const.tile([128, N_GROUPS, BATCH], bf16, tag="xT")
            y_sb = const.tile([128, N_BLOCKS, BATCH], f32, tag="y_sb")

            dma_engines = [nc.sync, nc.scalar]

            # first sign chunk's descriptors go first so its data lands ASAP
            s_chunks = []
            off = 0
            for c, wb in enumerate(CHUNK_BLOCKS):
                w = wb * 128
                sc = const.tile([128, N_GROUPS, w], f8, tag=f"sT{c}")
                dma_engines[c % 2].dma_start(
                    sc[:],
                    sT_d[:, off : off + N_GROUPS * w].rearrange(
                        "p (g o) -> p g o", g=N_GROUPS
                    ),
                )
                off += N_GROUPS * w
                s_chunks.append(sc)
                if c == 0:
                    nc.scalar.dma_start(
                        xT[:], xT_d[:].rearrange("p (g b) -> p g b", g=N_GROUPS)
                    )

            # map block index -> (chunk, o-offset within chunk)
            blk_loc = []
            for c, wb in enumerate(CHUNK_BLOCKS):
                for j in range(wb):
                    blk_loc.append((c, j * 128))

            piece = 0
            for b in range(N_BLOCKS):
                c, oc = blk_loc[b]
                sc = s_chunks[c]
                ps = psum.tile([128, BATCH], f32, tag="ps", bufs=2)
                for g in range(N_GROUPS):
                    nc.tensor.matmul(
                        ps[:, :],
                        sc[:, g, oc : oc + 128],
                        xT[:, g, :],
                        start=(g == 0),
                        stop=(g == N_GROUPS - 1),
                    )
                nc.vector.tensor_copy(y_sb[:, b, :], ps[:, :])
                if piece < len(Y_PIECES) and b == Y_PIECES[piece][1] - 1:
                    lo, hi = Y_PIECES[piece]
                    dma_engines[piece % 2].dma_start(
                        y_d[:, lo * BATCH : hi * BATCH].rearrange(
                            "p (blk b) -> p blk b", blk=hi - lo
                        ),
                        y_sb[:, lo:hi, :],
                    )
                    piece += 1
    nc.finalize()
    return nc


def _pack_signs(signs_shard, scales_shard):
    """[O_SHARD, IN_F] +/-1 and [O_SHARD, N_GROUPS] -> prescaled(e3m4) image
    [128, IMG_F], per-chunk contiguous per partition, g-major within chunk,
    zero-padded to O_PAD output columns."""
    import ml_dtypes

    f8 = ml_dtypes.float8_e3m4
    w_full = np.zeros((O_PAD, IN_F), dtype=np.float32)
    w_full[:O_SHARD] = signs_shard.astype(np.float32) * np.repeat(
        scales_shard.astype(np.float32) * W_RESCALE, GROUP, axis=1
    )
    sT = w_full.T  # [IN_F, O_PAD]
    img = np.empty((128, IMG_F), dtype=f8)
    off = 0
    o0 = 0
    for wb in CHUNK_BLOCKS:
        w = wb * 128
        sub = sT[:, o0 : o0 + w].reshape(N_GROUPS, 128, w)
        img[:, off : off + N_GROUPS * w] = (
            sub.transpose(1, 0, 2).reshape(128, N_GROUPS * w).astype(f8)
        )
        off += N_GROUPS * w
        o0 += w
    return img


def _pack_x(x):
    """[BATCH, IN_F] f32 -> xT bf16 [128, N_GROUPS*BATCH] with 1/64 folded."""
    import ml_dtypes

    xt = (np.asarray(x, np.float32) / W_RESCALE).T  # [IN_F, BATCH]
    return np.ascontiguousarray(
        xt.reshape(N_GROUPS, 128, BATCH).transpose(1, 0, 2).reshape(128, -1)
    ).astype(ml_dtypes.bfloat16)


def _shard_inputs(x, scales, signs):
    scales_r = np.asarray(scales, np.float32).reshape(OUT_F, N_GROUPS)
    xT_img = _pack_x(x)
    in_maps = []
    for c in range(N_CORES):
        lo, hi = c * O_SHARD, (c + 1) * O_SHARD
        in_maps.append(
            {
                "xT": xT_img,
                "signsT": _pack_signs(signs[lo:hi], scales_r[lo:hi]),
            }
        )
    return in_maps


def _unshard_out(res):
    cols = []
    for i in range(N_CORES):
        arr = np.asarray(res.results[i]["y"], np.float32)  # [128, 352]
        y_core = arr.reshape(128, N_BLOCKS, BATCH).transpose(1, 0, 2).reshape(
            N_BLOCKS * 128, BATCH
        )[:O_SHARD]
        cols.append(y_core.T)  # [32, 1376]
    return np.ascontiguousarray(np.concatenate(cols, axis=1), dtype=np.float32)


def _run(x, scales, signs, trace=False, tmpdir=None):
    from concourse import bass_utils

    if not _nc_cache:
        _nc_cache.append(build_nc())
    nc = _nc_cache[0]
    in_maps = _shard_inputs(x, scales, signs)
    res = bass_utils.run_bass_kernel_spmd(
        nc, in_maps, list(range(N_CORES)), trace=trace, tmpdir=tmpdir
    )
    return _unshard_out(res), res


def kernel(x, scales, signs):
    out, _ = _run(x, scales, signs)
    return out


# revision 5
# speedup vs baseline: 1.0494x; 1.0494x over previous
"""BitLinear v6: fp8(e3m4) prescaled weights, transpose-free, dequant-free.

Host ships signsT pre-transposed and prescaled by scale*64, quantized to
fp8 e3m4 (4 mantissa bits -> ~1.5e-2 rel err, under the 2e-2 gate); the
1/64 is folded into x, which ships pre-transposed in bf16. Device does
per-block accumulating matmuls (lhsT = fp8 sign tile [128k, r<=128o],
rhs = bf16 xT [128k, 32b]) into psum [r, 32b]:
  yT[o,b] = sum_g (s*scale*64)[o,g].T @ (x/64)T[g,b]

v6: the 96-wide remainder block (whose matmuls lose FWL and run ~3x
slower) is processed FIRST, hidden under stream startup; graded sign
chunks (small first so PE starts early) alternate between the sync and
scalar HWDGE descriptor rings; y goes out in two pieces, the bulk early
on the idle vector ring, the last three blocks at the end.
"""

import numpy as np

BATCH = 32
IN_F = 4096
OUT_F = 11008
GROUP = 128
N_GROUPS = IN_F // GROUP  # 32
N_CORES = 8
O_SHARD = OUT_F // N_CORES  # 1376
N_BLOCKS = 11  # 10 full 128-wide + one 96-wide
BLK_ORDER = [10] + list(range(10))  # 96-wide block first
BLK_W = [96] + [128] * 10  # width at each order position
CHUNK_POS = [[0], [1], [2], [3, 4], [5, 6], [7, 8], [9, 10]]  # order positions
IMG_F = N_GROUPS * O_SHARD  # 44032 free bytes per partition (fp8)
W_RESCALE = 64.0  # lift scales into e3m4 normal range [0.25, 15.5)

_nc_cache = []


def build_nc():
    import concourse.bacc as bacc
    import concourse.mybir as mybir
    import concourse.tile as tile

    f32 = mybir.dt.float32
    bf16 = mybir.dt.bfloat16
    f8 = mybir.dt.float8e3

    nc = bacc.Bacc(None, target_bir_lowering=False)
    xT_d = nc.dram_tensor("xT", [128, N_GROUPS * BATCH], bf16, kind="ExternalInput")
    sT_d = nc.dram_tensor("signsT", [128, IMG_F], f8, kind="ExternalInput")
    y_d = nc.dram_tensor("y", [128, N_BLOCKS * BATCH], f32, kind="ExternalOutput")

    with tile.TileContext(nc) as tc:
        with tc.tile_pool(name="const", bufs=1) as const, tc.tile_pool(
            name="psum", bufs=1, space="PSUM"
        ) as psum:
            xT = const.tile([128, N_GROUPS, BATCH], bf16, tag="xT")
            y_sb = const.tile([128, N_BLOCKS, BATCH], f32, tag="y_sb")

            nc.sync.dma_start(xT[:], xT_d[:].rearrange("p (g b) -> p g b", g=N_GROUPS))
            nc.vector.memset(y_sb[96:128, 0, :], 0.0)  # 96-block pad rows

            dma_engines = [nc.scalar, nc.sync]
            s_chunks = []  # (tile, base order-position)
            off = 0
            for c, poss in enumerate(CHUNK_POS):
                w = sum(BLK_W[p] for p in poss)
                sc = const.tile([128, N_GROUPS, w], f8, tag=f"sT{c}")
                dma_engines[c % 2].dma_start(
                    sc[:],
                    sT_d[:, off : off + N_GROUPS * w].rearrange(
                        "p (g o) -> p g o", g=N_GROUPS
                    ),
                )
                off += N_GROUPS * w
                s_chunks.append(sc)

            # order position -> (chunk idx, o-offset within chunk)
            pos_loc = {}
            for c, poss in enumerate(CHUNK_POS):
                o = 0
                for p in poss:
                    pos_loc[p] = (c, o)
                    o += BLK_W[p]

            for p in range(N_BLOCKS):
                c, oc = pos_loc[p]
                r = BLK_W[p]
                sc = s_chunks[c]
                ps = psum.tile([128, BATCH], f32, tag="ps", bufs=2)
                for g in range(N_GROUPS):
                    nc.tensor.matmul(
                        ps[:r, :],
                        sc[:, g, oc : oc + r],
                        xT[:, g, :],
                        start=(g == 0),
                        stop=(g == N_GROUPS - 1),
                    )
                nc.vector.tensor_copy(y_sb[:r, p, :], ps[:r, :])
                if p == 7:
                    nc.scalar.dma_start(
                        y_d[:, 0 : 8 * BATCH].rearrange(
                            "p (blk b) -> p blk b", blk=8
                        ),
                        y_sb[:, 0:8, :],
                    )
            nc.sync.dma_start(
                y_d[:, 8 * BATCH :].rearrange("p (blk b) -> p blk b", blk=3),
                y_sb[:, 8:11, :],
            )
    nc.finalize()
    return nc


def _pack_signs(signs_shard, scales_shard):
    """[O_SHARD, IN_F] +/-1 and [O_SHARD, N_GROUPS] -> prescaled(e3m4) image
    [128, IMG_F]; o-columns permuted into BLK_ORDER, per-chunk contiguous
    per partition, g-major within chunk."""
    import ml_dtypes

    f8 = ml_dtypes.float8_e3m4
    w_full = signs_shard.astype(np.float32) * np.repeat(
        scales_shard.astype(np.float32) * W_RESCALE, GROUP, axis=1
    )
    sT = w_full.T.astype(f8)  # [IN_F, O_SHARD]
    img = np.empty((128, IMG_F), dtype=f8)
    off = 0
    for poss in CHUNK_POS:
        cols = np.concatenate(
            [
                sT[:, BLK_ORDER[p] * 128 : BLK_ORDER[p] * 128 + BLK_W[p]]
                for p in poss
            ],
            axis=1,
        )
        w = cols.shape[1]
        img[:, off : off + N_GROUPS * w] = (
            cols.reshape(N_GROUPS, 128, w).transpose(1, 0, 2).reshape(128, -1)
        )
        off += N_GROUPS * w
    return img


def _pack_x(x):
    """[BATCH, IN_F] f32 -> xT bf16 [128, N_GROUPS*BATCH] with 1/64 folded."""
    import ml_dtypes

    xt = (np.asarray(x, np.float32) / W_RESCALE).T  # [IN_F, BATCH]
    return np.ascontiguousarray(
        xt.reshape(N_GROUPS, 128, BATCH).transpose(1, 0, 2).reshape(128, -1)
    ).astype(ml_dtypes.bfloat16)


def _shard_inputs(x, scales, signs):
    scales_r = np.asarray(scales, np.float32).reshape(OUT_F, N_GROUPS)
    xT_img = _pack_x(x)
    in_maps = []
    for c in range(N_CORES):
        lo, hi = c * O_SHARD, (c + 1) * O_SHARD
        in_maps.append(
            {
                "xT": xT_img,
                "signsT": _pack_signs(signs[lo:hi], scales_r[lo:hi]),
            }
        )
    return in_maps


def _unshard_out(res):
    cols = []
    for i in range(N_CORES):
        arr = np.asarray(res.results[i]["y"], np.float32)  # [128, 352]
        blocks = arr.reshape(128, N_BLOCKS, BATCH)
        y_core = np.empty((O_SHARD, BATCH), np.float32)
        for p in range(N_BLOCKS):
            b = BLK_ORDER[p]
            y_core[b * 128 : b * 128 + BLK_W[p]] = blocks[: BLK_W[p], p, :]
        cols.append(y_core.T)  # [32, 1376]
    return np.ascontiguousarray(np.concatenate(cols, axis=1), dtype=np.float32)


def _run(x, scales, signs, trace=False, tmpdir=None):
    from concourse import bass_utils

    if not _nc_cache:
        _nc_cache.append(build_nc())
    nc = _nc_cache[0]
    in_maps = _shard_inputs(x, scales, signs)
    res = bass_utils.run_bass_kernel_spmd(
        nc, in_maps, list(range(N_CORES)), trace=trace, tmpdir=tmpdir
    )
    return _unshard_out(res), res


def kernel(x, scales, signs):
    out, _ = _run(x, scales, signs)
    return out
